# revision 1
# baseline (speedup 1.0000x reference)
"""GAT layer kernel for Trainium2, SPMD over 8 NeuronCores.

Reference computation (per batch b):
  h  = x @ W_lin.T                          [N, O]
  hp = concat(h, prior[None, :])            [N1, O]
  per head: hp_h = hp @ w_head[h]           [N1, O]
  t = tanh(hp_h); s_src = t @ a_src[h]; s_dst = t @ a_dst[h]
  z[i,j] = s_src[i] + s_dst[j]; y = leaky_relu(z, 0.2)
  y[mask_i | mask_j] = -1e18; p = softmax_j(y)
  out_h = p @ hp_h;  out = mean_h(out_h) + bias

Sharding: core c handles batch b=c//2 and heads h in {2*(c%2), 2*(c%2)+1}.
Each core computes, for its two heads, the transposed partial output
  outT[h] = (0.25 / sum_j e[j,i]) * sum_j hp_h[j,:] * e[j,i]   in [O, N1]
entirely on-chip (flash style, no N1xN1 slab in DRAM). The host adds the
two heads of the two cores per batch and transposes.

Softmax is computed without max subtraction (scores are bounded by ~30 in
magnitude since |s| <= ||tanh|| * ||a||), with the mask folded into the
score vectors (sentinel -400, see NEG below):
  - masked j (column): s_dst'[j] ~ -400 -> e ~ e^-80 ~ 0
  - masked i (row): whole row e ~ 0; a rank-1 correction (vbar x m_row
    added to the PE accumulation, +m added to the sums) reproduces the
    reference's uniform-attention rows exactly.
The kernel returns the unnormalized accumulations and the softmax
denominators; the host divides, averages heads, transposes, adds bias.
"""

import sys

for _p in ("/opt/trn_rl_repo",):
    if _p not in sys.path:
        sys.path.insert(0, _p)

import numpy as np

import concourse.bass as bass
import concourse.tile as tile
from concourse import bacc, mybir
from concourse.masks import make_identity

FP = mybir.dt.float32
U8 = mybir.dt.uint8
N, N1, I, O = 2047, 2048, 256, 128
HPC = 2  # heads per core
NCORES = 8
# Mask sentinel. The ACT exp table only accepts inputs in ~[-87.3, 88.7],
# so we cannot use -1e18 like the reference. With the mask folded into the
# score vectors BEFORE leaky-relu, a masked score z ~ -400 becomes
# y = 0.2*z ~ -80 after the leaky slope: in-range for exp, and e^-80 is
# ~1e-35 -- negligible vs. any row sum (>= e^-6). Inputs that could still
# leave the table range (double-masked pairs, the un-slope'd exp(z) in
# route B) are clamped; clamped values contribute < 1e-12 absolute.
NEG = -400.0
ZCLAMP = -425.0   # route A: z = max(z, ZCLAMP) -> y >= -85
SCLAMP = -43.0    # route B: per-vector clamp -> e1/e2 inputs >= -86
Tanh = mybir.ActivationFunctionType.Tanh
Exp = mybir.ActivationFunctionType.Exp
ALU = mybir.AluOpType

# jc indices whose leaky-relu runs on DVE (route A); the rest run the
# two-exp route (B) on ACT. Tuned for ACT/DVE balance.
import os as _os2
if _os2.environ.get("GAT_A_ALL"):
    A_SET = frozenset(range(16))
elif _os2.environ.get("GAT_B_ALL"):
    A_SET = frozenset()
else:
    A_SET = frozenset(range(0, 14, 2))
NO_EXP = bool(_os2.environ.get("GAT_NO_EXP"))
NO_TTR = bool(_os2.environ.get("GAT_NO_TTR"))
NO_TS2 = bool(_os2.environ.get("GAT_NO_TS2"))
# bf16 attention weights: halves the PE stream cost (fp32 streams at
# ~2 cycles/column). Softmax weights appear in both numerator and
# denominator, so most of the bf16 rounding cancels.
BF16_ET = bool(_os2.environ.get("GAT_BF16"))
ET_DT = mybir.dt.bfloat16 if BF16_ET else FP

# debug bisection stage: 1=prep, 2=+head prep, 25=+av, 26=+sums,
# 27=+correction, 99=full (default)
import os as _os
STAGE = int(_os.environ.get("GAT_KERNEL_STAGE", "99"))


def g5(g):
    return slice(g * 512, (g + 1) * 512)


def c128(c):
    return slice(c * 128, (c + 1) * 128)


def _build() -> bass.Bass:
    nc = bacc.Bacc(None, target_bir_lowering=False, debug=False)
    x_b = nc.dram_tensor("x_b", [N, I], FP, kind="ExternalInput")
    prior_b = nc.dram_tensor("prior_b", [O], FP, kind="ExternalInput")
    mask_b = nc.dram_tensor("mask_b", [N1], U8, kind="ExternalInput")
    W_lin = nc.dram_tensor("W_lin", [O, I], FP, kind="ExternalInput")
    w_pair = nc.dram_tensor("w_pair", [HPC, O, O], FP, kind="ExternalInput")
    a_src_p = nc.dram_tensor("a_src_p", [HPC, O], FP, kind="ExternalInput")
    a_dst_p = nc.dram_tensor("a_dst_p", [HPC, O], FP, kind="ExternalInput")
    outT = nc.dram_tensor("outT", [HPC, O, N1], FP, kind="ExternalOutput")
    sums = nc.dram_tensor("sums", [HPC, N1], FP, kind="ExternalOutput")
    sdst_dram = nc.dram_tensor("sdst_scratch", [N1], FP)

    with tile.TileContext(nc) as tc:
        with (
            tc.tile_pool(name="constp", bufs=1) as constp,
            tc.tile_pool(name="bigp", bufs=1) as bigp,
            tc.tile_pool(name="headp", bufs=1) as headp,
            tc.tile_pool(name="scratch", bufs=6) as scratch,
            tc.tile_pool(name="outp", bufs=2) as outp,
            tc.tile_pool(name="pp", bufs=2, space="PSUM") as pp,
            tc.tile_pool(name="pav", bufs=1, space="PSUM") as pav,
            tc.tile_pool(name="psums", bufs=1, space="PSUM") as psums,
        ):
            pools = dict(constp=constp, bigp=bigp, headp=headp,
                         scratch=scratch, outp=outp, pp=pp,
                         pav=pav, psums=psums, tc=tc)
            _body(nc, tc, pools,
                  x_b, prior_b, mask_b, W_lin, w_pair, a_src_p, a_dst_p,
                  outT, sums, sdst_dram)
    return nc


def _body(nc, tc, pools,
          x_b, prior_b, mask_b, W_lin, w_pair, a_src_p, a_dst_p,
          outT, sums, sdst_dram):
    constp, bigp, headp = pools["constp"], pools["bigp"], pools["headp"]
    scratch, outp = pools["scratch"], pools["outp"]
    pp, pav, psums = pools["pp"], pools["pav"], pools["psums"]
    tcx = pools["tc"]

    # ---- constants ----
    ident = constp.tile([128, 128], FP, tag="ident")
    make_identity(nc, ident)
    ones_row = constp.tile([1, 128], FP, tag="ones_row")
    nc.vector.memset(ones_row, 1.0)
    quarter_row = constp.tile([1, 128], FP, tag="quarter_row")
    nc.vector.memset(quarter_row, 0.25)
    one_one = constp.tile([1, 1], FP, tag="one_one")
    nc.vector.memset(one_one, 1.0)
    ones_col = constp.tile([128, 1], FP, tag="ones_col")
    nc.vector.memset(ones_col, 1.0)
    ones_col_et = constp.tile([128, 1], ET_DT, tag="ones_col_et")
    nc.vector.memset(ones_col_et, 1.0)

    # mask rows in f32: m_row and -1e18 * m broadcast to 2 partitions
    m2u8 = constp.tile([2, N1], U8, tag="m2u8")
    nc.sync.dma_start(out=m2u8[0:1, :], in_=mask_b[None, :])
    nc.sync.dma_start(out=m2u8[1:2, :], in_=mask_b[None, :])
    m_row = constp.tile([1, N1], FP, tag="m_row")
    nc.vector.tensor_copy(m_row, m2u8[0:1, :])
    negm2 = constp.tile([2, N1], FP, tag="negm2")
    nc.vector.tensor_scalar(negm2, m2u8, NEG, None, op0=ALU.mult)

    hpT = bigp.tile([128, N1], FP, tag="hpT")
    with tcx.tile_pool(name="prep", bufs=1) as prep:
        # ---- W_lin transposed: wlT[:, k, :] = W_lin[:, k*128:...].T ----
        wl = prep.tile([128, I], FP, tag="wl", bufs=1)
        nc.sync.dma_start(out=wl, in_=W_lin[:, :])
        wlT = prep.tile([128, 2, 128], FP, tag="wlT", bufs=1)
        for k in range(2):
            ps = pp.tile([128, 512], FP, tag="tr")
            nc.tensor.transpose(ps[:, :128], wl[:, c128(k)], ident)
            nc.vector.tensor_copy(wlT[:, k, :], ps[:, :128])

        # ---- x transposed: xT[:, k, n] = x[n, k*128 + i] ----
        # (last tile has 127 real rows; row 127 is zeroed so a full
        # 128-row transpose lands zeros in xT column 2047, later
        # overwritten by prior)
        xT = prep.tile([128, 2, N1], FP, tag="xT", bufs=1)
        for t in range(16):
            rows = 128 if t < 15 else 127
            xn = prep.tile([128, I], FP, tag="xn", bufs=3)
            if rows < 128:
                nc.vector.memset(xn, 0.0)
            nc.sync.dma_start(out=xn[:rows, :],
                              in_=x_b[t * 128: t * 128 + rows, :])
            for k in range(2):
                ps = pp.tile([128, 512], FP, tag="tr")
                nc.tensor.transpose(ps[:, :128], xn[:, c128(k)], ident)
                nc.vector.tensor_copy(xT[:, k, t * 128: (t + 1) * 128],
                                      ps[:, :128])

        # ---- hpT[o, n] = (x @ W_lin.T).T, col N-1..N1-1 = prior ----
        for g in range(4):
            ph = pp.tile([128, 512], FP, tag="tr")
            for k in range(2):
                nc.tensor.matmul(ph, wlT[:, k, :], xT[:, k, g5(g)],
                                 start=(k == 0), stop=(k == 1))
            nc.vector.tensor_copy(hpT[:, g5(g)], ph)
        nc.sync.dma_start(out=hpT[:, 2047:2048], in_=prior_b[:, None])

    with tcx.tile_pool(name="etp", bufs=5) as etp:

        # column sums of hp (for the cheap per-head vbar = hpbar @ w_head)
        hpbar_col = constp.tile([128, 1], FP, tag="hpbar_col")
        nc.vector.reduce_sum(hpbar_col, hpT, axis=mybir.AxisListType.X)

        if STAGE == 1:
            nc.sync.dma_start(out=outT[0, :, :], in_=hpT)
            return

        for h in range(HPC):
            # ---- head weights ----
            wh = headp.tile([128, 128], FP, tag="wh")
            nc.sync.dma_start(out=wh, in_=w_pair[h])
            acols = headp.tile([128, 2], FP, tag="acols")
            nc.sync.dma_start(out=acols[:, 0:1], in_=a_src_p[h][:, None])
            nc.sync.dma_start(out=acols[:, 1:2], in_=a_dst_p[h][:, None])

            # ---- tanh(hp_h.T) and masked score vectors s2' = [s_src'; s_dst'] ----
            tT = bigp.tile([128, N1], FP, tag="tT")
            for g in range(4):
                php = pp.tile([128, 512], FP, tag="tr")
                nc.tensor.matmul(php, wh, hpT[:, g5(g)], start=True, stop=True)
                nc.scalar.activation(tT[:, g5(g)], php, Tanh)
            s2 = headp.tile([2, N1], FP, tag="s2")
            for g in range(4):
                ps2 = pp.tile([128, 512], FP, tag="tr")
                nc.tensor.matmul(ps2[:2, :], acols, tT[:, g5(g)],
                                 start=True, stop=True)
                nc.vector.tensor_tensor(s2[:, g5(g)], ps2[:2, :],
                                        negm2[:, g5(g)], op=ALU.add)

            # ---- V = hp_h (natural [n, p]) and vbar = mean_n V ----
            V = bigp.tile([128, N1], ET_DT, tag="V")
            for t in range(16):
                pv = pp.tile([128, 512], FP, tag="tr")
                nc.tensor.matmul(pv[:, :128], hpT[:, c128(t)], wh,
                                 start=True, stop=True)
                nc.vector.tensor_copy(V[:, c128(t)], pv[:, :128])
            pvb = pp.tile([128, 512], FP, tag="tr")
            nc.tensor.matmul(pvb[:1, :128], hpbar_col, wh, start=True, stop=True)
            vbar = headp.tile([1, 128], FP, tag="vbar")
            nc.vector.tensor_scalar_mul(vbar, pvb[:1, :128], 1.0 / N1)

            # ---- srcb[p, i] = s_src'[i] (broadcast over partitions) ----
            srcb = bigp.tile([128, N1], FP, tag="srcb")
            for g in range(4):
                pb = pp.tile([128, 512], FP, tag="tr")
                nc.tensor.matmul(pb, ones_row, s2[0:1, g5(g)],
                                 start=True, stop=True)
                nc.vector.tensor_copy(srcb[:, g5(g)], pb)

            # ---- s_dst' as columns via DRAM bounce ----
            nc.sync.dma_start(out=sdst_dram[:], in_=s2[1:2, :])
            sdc = headp.tile([128, 16], FP, tag="sdc")
            nc.sync.dma_start(out=sdc,
                              in_=sdst_dram[:].rearrange("(c p) -> p c", p=128))
            # clamped variants for route B (exp-table range safety)
            sdc_c = headp.tile([128, 16], FP, tag="sdc_c")
            nc.vector.tensor_scalar_max(sdc_c, sdc, SCLAMP)
            sdc02c = headp.tile([128, 16], FP, tag="sdc02c")
            nc.vector.tensor_scalar(sdc02c, sdc, 0.2, SCLAMP,
                                    op0=ALU.mult, op1=ALU.max)
            srcb_c = bigp.tile([128, N1], FP, tag="srcb_c")
            nc.vector.tensor_scalar_max(srcb_c, srcb, SCLAMP)
            srcb02c = bigp.tile([128, N1], FP, tag="srcb02c")
            nc.vector.tensor_scalar(srcb02c, srcb, 0.2, SCLAMP,
                                    op0=ALU.mult, op1=ALU.max)

            if STAGE == 2:
                nc.sync.dma_start(out=outT[h, :, :], in_=V)
                continue

            # ---- main loop over j-chunks ----
            av = pav.tile([128, N1], FP, tag="av")
            # 4 per-i-group row-sum accumulators, packed two per PSUM bank at
            # the legal matmul output partition bases (0 and 32).
            sumpA = psums.tile([33, 512], FP, tag="sumpA")
            sumpB = psums.tile([33, 512], FP, tag="sumpB")

            def sum_slot(g):
                t = sumpA if g < 2 else sumpB
                base = 32 * (g % 2)
                return t[base:base + 1, :]
            for jc in range(16):
                col = sdc[:, jc:jc + 1]
                eT = etp.tile([128, N1], ET_DT, tag="eT")
                if jc in A_SET:
                    # route A: leaky-relu on DVE: y = 0.2*(z + max(4z, 0))
                    z = scratch.tile([128, N1], FP, tag="scr")
                    if NO_TS2:
                        nc.vector.tensor_scalar(z, srcb, col, None, op0=ALU.add)
                    else:
                        nc.vector.tensor_scalar(z, srcb, col, ZCLAMP,
                                                op0=ALU.add, op1=ALU.max)
                    r4 = scratch.tile([128, N1], FP, tag="scr")
                    nc.vector.tensor_scalar(r4, z, 4.0, 0.0,
                                            op0=ALU.mult, op1=ALU.max)
                    # y4 = z + max(4z,0) = 5*lrelu(z); the 0.2 folds into the
                    # activation's input scale: e = exp(0.2 * y4)
                    y4 = scratch.tile([128, N1], FP, tag="scr")
                    nc.vector.tensor_tensor(y4, z, r4, op=ALU.add)
                    nc.scalar.activation(eT, y4,
                                         mybir.ActivationFunctionType.Identity
                                         if NO_EXP else Exp, scale=0.2)
                else:
                    # route B: e = max(exp(z), exp(0.2 z)), builds fused in ACT,
                    # with clamped operands so exp inputs stay in table range
                    e1 = scratch.tile([128, N1], FP, tag="scr")
                    nc.scalar.activation(e1, srcb_c, Exp,
                                         bias=sdc_c[:, jc:jc + 1], scale=1.0)
                    e2 = scratch.tile([128, N1], FP, tag="scr")
                    nc.scalar.activation(e2, srcb02c, Exp,
                                         bias=sdc02c[:, jc:jc + 1], scale=1.0)
                    nc.vector.tensor_tensor(eT, e1, e2, op=ALU.max)
                if STAGE == 21:
                    if jc == 15:
                        nc.sync.dma_start(out=outT[h, :, :], in_=eT)
                    continue
                for g in range(4):
                    nc.tensor.matmul(av[:, g5(g)], V[:, c128(jc)], eT[:, g5(g)],
                                     start=(jc == 0), stop=(STAGE < 27 and jc == 15),
                                     skip_group_check=True)
                if STAGE >= 26:
                    for g in range(4):
                        nc.tensor.matmul(sum_slot(g), ones_col_et, eT[:, g5(g)],
                                         start=(jc == 0), stop=(STAGE < 27 and jc == 15),
                                         skip_group_check=True)

            if STAGE >= 27:
                # ---- masked-row correction: av += vbar x m, sum += m ----
                for g in range(4):
                    nc.tensor.matmul(sum_slot(g), one_one, m_row[:, g5(g)],
                                     start=False, stop=True, skip_group_check=True)
                for g in range(4):
                    nc.tensor.matmul(av[:, g5(g)], vbar, m_row[:, g5(g)],
                                     start=False, stop=True, skip_group_check=True)

            if STAGE >= 28:
                # ---- export unnormalized av and the sums; host divides ----
                sum_sb = headp.tile([1, N1], FP, tag="sum_sb")
                for g in range(4):
                    nc.vector.tensor_copy(sum_sb[:, g5(g)], sum_slot(g))
                nc.sync.dma_start(out=sums[h, :], in_=sum_sb)
                for g in range(4):
                    outF = outp.tile([128, 512], FP, tag="outF")
                    nc.vector.tensor_copy(outF, av[:, g5(g)])
                    nc.sync.dma_start(out=outT[h, :, g5(g)], in_=outF)
            elif STAGE >= 25:
                for g in range(4):
                    outF = outp.tile([128, 512], FP, tag="outF")
                    nc.vector.tensor_copy(outF, av[:, g5(g)])
                    nc.sync.dma_start(out=outT[h, :, g5(g)], in_=outF)


_NC_CACHE = None


def _get_nc():
    global _NC_CACHE
    if _NC_CACHE is None:
        nc = _build()
        nc.finalize()
        _NC_CACHE = nc
    return _NC_CACHE


def make_in_maps(x, prior_feature, x_mask, W_lin, w_head, a_src, a_dst):
    x = np.ascontiguousarray(np.asarray(x, np.float32))
    prior_feature = np.ascontiguousarray(np.asarray(prior_feature, np.float32))
    x_mask_u8 = np.ascontiguousarray(np.asarray(x_mask).astype(np.uint8))
    W_lin = np.ascontiguousarray(np.asarray(W_lin, np.float32))
    w_head = np.ascontiguousarray(np.asarray(w_head, np.float32))
    a_src = np.ascontiguousarray(np.asarray(a_src, np.float32))
    a_dst = np.ascontiguousarray(np.asarray(a_dst, np.float32))
    in_maps = []
    for c in range(NCORES):
        b, h0 = c // 2, (c % 2) * HPC
        in_maps.append(dict(
            x_b=x[b],
            prior_b=prior_feature[b],
            mask_b=x_mask_u8[b],
            W_lin=W_lin,
            w_pair=np.ascontiguousarray(w_head[h0:h0 + HPC]),
            a_src_p=np.ascontiguousarray(a_src[h0:h0 + HPC]),
            a_dst_p=np.ascontiguousarray(a_dst[h0:h0 + HPC]),
        ))
    return in_maps


def combine_results(results, bias):
    out = np.zeros((4, N1, O), np.float32)
    for c in range(NCORES):
        b = c // 2
        o = results[c]["outT"]    # [HPC, O, N1] unnormalized
        s = results[c]["sums"]    # [HPC, N1] softmax denominators
        out[b] += (o[0] / s[0][None, :] + o[1] / s[1][None, :]).T * 0.25
    out += np.asarray(bias, np.float32)[None, None, :]
    return out


def kernel(x, prior_feature, x_mask, W_lin, w_head, a_src, a_dst, bias,
           **run_kwargs):
    from concourse.bass_utils import run_bass_kernel_spmd
    nc = _get_nc()
    in_maps = make_in_maps(x, prior_feature, x_mask, W_lin, w_head,
                           a_src, a_dst)
    br = run_bass_kernel_spmd(nc, in_maps, core_ids=list(range(NCORES)),
                              **run_kwargs)
    out = combine_results(br.results, bias)
    if run_kwargs:
        kernel.last_bass_results = br
    return out



# revision 12
# speedup vs baseline: 1.5774x; 1.5774x over previous
"""GAT layer kernel for Trainium2, SPMD over 8 NeuronCores.

Reference computation (per batch b):
  h  = x @ W_lin.T                          [N, O]
  hp = concat(h, prior[None, :])            [N1, O]
  per head: hp_h = hp @ w_head[h]           [N1, O]
  t = tanh(hp_h); s_src = t @ a_src[h]; s_dst = t @ a_dst[h]
  z[i,j] = s_src[i] + s_dst[j]; y = leaky_relu(z, 0.2)
  y[mask_i | mask_j] = -1e18; p = softmax_j(y)
  out_h = p @ hp_h;  out = mean_h(out_h) + bias

Sharding: core c handles batch b=c//2 and heads h in {2*(c%2), 2*(c%2)+1}.

Key algebra: exp(leaky_relu(z)) = max(exp(z), exp(0.2 z)) and z factors as
s_src[i] + s_dst[j].  Softmax is invariant to any positive per-i scale, so
dividing by exp(0.2 s_src[i]) gives the unnormalized weights
  e[j,i] = max(r[i] * v[j], v2[j])
with r = exp(0.8 s_src'), v = exp(s_dst'), v2 = exp(0.2 s_dst'), where the
primed scores carry the mask sentinel (-400, clamped to the exp table
range).  Each 128-row j-chunk of e is then ONE fused DVE tensor_scalar
(mult, max) producing bf16 weights directly; the PE accumulates
  avT[o, i] = sum_j V[j, o] e[j, i]   and   sums[i] = sum_j e[j, i]
flash-style in PSUM.  The host divides, fixes fully-masked rows i (whole
row masked -> reference softmax is uniform -> out row = mean_n hp_h = vbar,
exported per head), averages heads, transposes, adds bias.

fp32 operands of the remaining matmuls are bitcast to float32r (TF32-like,
1 cycle/row for free-size >= 256 vs 4 for fp32).
"""

import sys

for _p in ("/opt/trn_rl_repo",):
    if _p not in sys.path:
        sys.path.insert(0, _p)

import os as _os

import numpy as np

import concourse.bass as bass
import concourse.tile as tile
from concourse import bacc, mybir
from concourse.masks import make_identity

FP = mybir.dt.float32
FPR = mybir.dt.float32r
BF = mybir.dt.bfloat16
U8 = mybir.dt.uint8
N, N1, I, O = 2047, 2048, 256, 128
HPC = 2  # heads per core
NCORES = 8
# Mask sentinel: s' = s - 400 for masked nodes.  exp-table inputs are kept
# inside ~[-87, 88]: v2 = exp(0.2 s') >= exp(-82) needs no clamp; v and r
# inputs are clamped at CLO.  Clamped weights are ~e^-86 ~ 4e-38, vs >=
# ~e^-16 for any live entry -- negligible.
NEG = -400.0
CLO = -86.0
Tanh = mybir.ActivationFunctionType.Tanh
Exp = mybir.ActivationFunctionType.Exp
Ident = mybir.ActivationFunctionType.Identity
ALU = mybir.AluOpType

USE_FPR = not bool(_os.environ.get("GATV2_NO_FPR"))
MMDT = FPR if USE_FPR else FP
TS_FUSED = not bool(_os.environ.get("GATV2_TS1"))
ACT_COPY = not bool(_os.environ.get("GATV2_NO_ACTCOPY"))
STAGE = int(_os.environ.get("GATV2_STAGE", "99"))


def g5(g):
    return slice(g * 512, (g + 1) * 512)


def c128(c):
    return slice(c * 128, (c + 1) * 128)


def _build() -> bass.Bass:
    nc = bacc.Bacc(None, target_bir_lowering=False, debug=False)
    x_b = nc.dram_tensor("x_b", [N, I], FP, kind="ExternalInput")
    prior_b = nc.dram_tensor("prior_b", [O], FP, kind="ExternalInput")
    mask_b = nc.dram_tensor("mask_b", [N1], U8, kind="ExternalInput")
    W_lin = nc.dram_tensor("W_lin", [O, I], FP, kind="ExternalInput")
    w_pair = nc.dram_tensor("w_pair", [HPC, O, O], FP, kind="ExternalInput")
    a_src_p = nc.dram_tensor("a_src_p", [HPC, O], FP, kind="ExternalInput")
    a_dst_p = nc.dram_tensor("a_dst_p", [HPC, O], FP, kind="ExternalInput")
    outT = nc.dram_tensor("outT", [HPC, O, N1], FP, kind="ExternalOutput")
    sums = nc.dram_tensor("sums", [HPC, N1], FP, kind="ExternalOutput")
    vbar_out = nc.dram_tensor("vbar_out", [HPC, O], FP, kind="ExternalOutput")
    sdst_dram = nc.dram_tensor("sdst_scratch", [HPC, N1], FP)
    dbg = (nc.dram_tensor("dbg", [HPC, 128, 10496], FP, kind="ExternalOutput")
           if STAGE == 3 else None)

    with tile.TileContext(nc) as tc:
        with (
            tc.tile_pool(name="constp", bufs=1) as constp,
            tc.tile_pool(name="bigp", bufs=1) as bigp,
            tc.tile_pool(name="headp", bufs=2) as headp,
            tc.tile_pool(name="etp", bufs=5 if TS_FUSED else 3) as etp,
            tc.tile_pool(name="outp", bufs=2) as outp,
            tc.tile_pool(name="pp", bufs=2, space="PSUM") as pp,
            tc.tile_pool(name="pav", bufs=1, space="PSUM") as pav,
            tc.tile_pool(name="psums", bufs=1, space="PSUM") as psums,
        ):
            pools = dict(constp=constp, bigp=bigp, headp=headp, etp=etp,
                         outp=outp, pp=pp, pav=pav, psums=psums, tc=tc)
            _body(nc, tc, pools,
                  x_b, prior_b, mask_b, W_lin, w_pair, a_src_p, a_dst_p,
                  outT, sums, vbar_out, sdst_dram, dbg)
    return nc


def _body(nc, tc, pools,
          x_b, prior_b, mask_b, W_lin, w_pair, a_src_p, a_dst_p,
          outT, sums, vbar_out, sdst_dram, dbg=None):
    constp, bigp, headp = pools["constp"], pools["bigp"], pools["headp"]
    etp, outp = pools["etp"], pools["outp"]
    pp, pav, psums = pools["pp"], pools["pav"], pools["psums"]
    tcx = pools["tc"]

    def ccopy(dst, srcap):
        if ACT_COPY:
            nc.scalar.activation(dst, srcap, Ident)
        else:
            nc.vector.tensor_copy(dst, srcap)

    # ---- constants ----
    ident = constp.tile([128, 128], FP, tag="ident")
    make_identity(nc, ident)
    ones_row = constp.tile([1, 128], FP, tag="ones_row")
    nc.vector.memset(ones_row, 1.0)
    ones_row_r = constp.tile([1, 128], MMDT, tag="ones_row_r")
    ccopy(ones_row_r, ones_row)
    ones_col_bf = constp.tile([128, 1], BF, tag="ones_col_bf")
    nc.vector.memset(ones_col_bf, 1.0)

    # masked-node sentinel rows: negm2[{src,dst}, :] = NEG * mask
    m2u8 = constp.tile([2, N1], U8, tag="m2u8")
    nc.sync.dma_start(out=m2u8[0:1, :], in_=mask_b[None, :])
    nc.sync.dma_start(out=m2u8[1:2, :], in_=mask_b[None, :])
    negm2 = constp.tile([2, N1], FP, tag="negm2")
    nc.vector.tensor_scalar(negm2, m2u8, NEG, None, op0=ALU.mult)

    # hpT in fp32 (for the fp32 V matmuls) and fp32r (for tT/s2 streams)
    hpT = bigp.tile([128, N1], FP, tag="hpT")
    hpT_r = bigp.tile([128, N1], MMDT, tag="hpT_r")
    with tcx.tile_pool(name="prep", bufs=1) as prep:
        # ---- W_lin transposed: wlT[:, k, :] = W_lin[:, k*128:...].T ----
        wl = prep.tile([128, I], FP, tag="wl", bufs=1)
        nc.sync.dma_start(out=wl, in_=W_lin[:, :])
        wlT = prep.tile([128, 2, 128], MMDT, tag="wlT", bufs=1)
        for k in range(2):
            ps = pp.tile([128, 512], FP, tag="tr")
            nc.tensor.transpose(ps[:, :128], wl[:, c128(k)], ident)
            nc.vector.tensor_copy(wlT[:, k, :], ps[:, :128])

        # ---- x transposed: xT[:, k, n] = x[n, k*128 + i] ----
        # (row 127 of the last tile is zeroed; xT column 2047 is later
        # overwritten by prior)
        xT = prep.tile([128, 2, N1], MMDT, tag="xT", bufs=1)
        for t in range(16):
            rows = 128 if t < 15 else 127
            xn = prep.tile([128, I], FP, tag="xn", bufs=3)
            if rows < 128:
                nc.vector.memset(xn, 0.0)
            nc.sync.dma_start(out=xn[:rows, :],
                              in_=x_b[t * 128: t * 128 + rows, :])
            for k in range(2):
                ps = pp.tile([128, 512], FP, tag="tr")
                nc.tensor.transpose(ps[:, :128], xn[:, c128(k)], ident)
                nc.vector.tensor_copy(xT[:, k, t * 128: (t + 1) * 128],
                                      ps[:, :128])

        # ---- hpT[o, n] = (x @ W_lin.T).T, col 2047 = prior ----
        prior_sb = prep.tile([128, 1], FP, tag="prior_sb", bufs=1)
        nc.sync.dma_start(out=prior_sb, in_=prior_b[:, None])
        for g in range(4):
            ph = pp.tile([128, 512], FP, tag="tr")
            for k in range(2):
                nc.tensor.matmul(ph, wlT[:, k, :], xT[:, k, g5(g)],
                                 start=(k == 0), stop=(k == 1))
            nc.vector.tensor_copy(hpT[:, g5(g)], ph)
            ccopy(hpT_r[:, g5(g)], ph)
        nc.vector.tensor_copy(hpT[:, 2047:2048], prior_sb)
        ccopy(hpT_r[:, 2047:2048], prior_sb)

    # column sums of hp (for vbar = (hpbar @ w_head) / N1)
    hpbar_col = constp.tile([128, 1], FP, tag="hpbar_col")
    nc.vector.reduce_sum(hpbar_col, hpT, axis=mybir.AxisListType.X)

    if STAGE == 1:
        nc.sync.dma_start(out=outT[0, :, :], in_=hpT)
        return

    for h in range(HPC):
        # ---- head weights ----
        wh = headp.tile([128, 128], FP, tag="wh")
        nc.sync.dma_start(out=wh, in_=w_pair[h])
        wh_r = headp.tile([128, 128], MMDT, tag="wh_r")
        ccopy(wh_r, wh)
        acols = headp.tile([128, 2], FP, tag="acols")
        nc.sync.dma_start(out=acols[:, 0:1], in_=a_src_p[h][:, None])
        nc.sync.dma_start(out=acols[:, 1:2], in_=a_dst_p[h][:, None])
        acols_r = headp.tile([128, 2], MMDT, tag="acols_r")
        ccopy(acols_r, acols)

        # ---- tanh(hp_h.T) and masked scores s2' = [s_src'; s_dst'] ----
        tT = headp.tile([128, N1], MMDT, tag="tT")
        for g in range(4):
            php = pp.tile([128, 512], FP, tag="tr")
            nc.tensor.matmul(php, wh_r, hpT_r[:, g5(g)],
                             start=True, stop=True)
            nc.scalar.activation(tT[:, g5(g)], php, Tanh)
        s2 = headp.tile([2, N1], FP, tag="s2")
        for g in range(4):
            ps2 = pp.tile([128, 512], FP, tag="tr")
            nc.tensor.matmul(ps2[:2, :], acols_r, tT[:, g5(g)],
                             start=True, stop=True)
            nc.vector.tensor_tensor(s2[:, g5(g)], ps2[:2, :],
                                    negm2[:, g5(g)], op=ALU.add)

        # ---- V = hp_h in natural [n, p] layout (bf16), and vbar ----
        V = headp.tile([128, N1], BF, tag="V")
        for t in range(16):
            pv = pp.tile([128, 512], FP, tag="tr")
            nc.tensor.matmul(pv[:, :128], hpT[:, c128(t)], wh,
                             start=True, stop=True)
            nc.vector.tensor_copy(V[:, c128(t)], pv[:, :128])
        pvb = pp.tile([128, 512], FP, tag="tr")
        nc.tensor.matmul(pvb[:1, :128], hpbar_col, wh, start=True, stop=True)
        vbar_sb = headp.tile([1, 128], FP, tag="vbar_sb")
        nc.vector.tensor_scalar_mul(vbar_sb, pvb[:1, :128], 1.0 / N1)
        nc.sync.dma_start(out=vbar_out[h, :], in_=vbar_sb)

        # ---- s_dst' as columns via DRAM bounce; v / v2 columns ----
        nc.sync.dma_start(out=sdst_dram[h, :], in_=s2[1:2, :])
        sdc = headp.tile([128, 16], FP, tag="sdc")
        nc.sync.dma_start(out=sdc,
                          in_=sdst_dram[h, :].rearrange("(c p) -> p c", p=128))
        sdc_c = headp.tile([128, 16], FP, tag="sdc_c")
        nc.vector.tensor_scalar_max(sdc_c, sdc, CLO)
        v_col = headp.tile([128, 16], FP, tag="v_col")
        nc.scalar.activation(v_col, sdc_c, Exp)
        # 0.2 * sdc >= -82: already inside the exp table range
        v2_col = headp.tile([128, 16], FP, tag="v2_col")
        nc.scalar.activation(v2_col, sdc, Exp, scale=0.2)

        # ---- r = exp(0.8 s_src') broadcast to all partitions via PE ----
        rr_tmp = headp.tile([1, N1], FP, tag="rr_tmp")
        nc.vector.tensor_scalar(rr_tmp, s2[0:1, :], 0.8, CLO,
                                op0=ALU.mult, op1=ALU.max)
        r_row = headp.tile([1, N1], MMDT, tag="r_row")
        nc.scalar.activation(r_row, rr_tmp, Exp)
        rb = headp.tile([128, N1], FP, tag="rb")
        for g in range(4):
            prb = pp.tile([128, 512], FP, tag="tr")
            nc.tensor.matmul(prb, ones_row_r, r_row[:, g5(g)],
                             start=True, stop=True)
            ccopy(rb[:, g5(g)], prb)

        if STAGE == 2:
            nc.sync.dma_start(out=outT[h, :, :], in_=rb)
            continue
        if STAGE == 3:
            nc.sync.dma_start(out=dbg[h, :, 0:2048], in_=rb)
            nc.sync.dma_start(out=dbg[h, :, 4096:4112], in_=sdc)
            nc.sync.dma_start(out=dbg[h, :, 4112:4128], in_=v_col)
            nc.sync.dma_start(out=dbg[h, :, 4128:4144], in_=v2_col)
            nc.sync.dma_start(out=dbg[h, :, 4144:6192],
                              in_=tT[:, :].bitcast(FP) if USE_FPR else tT[:, :])
            nc.sync.dma_start(out=dbg[h, :, 6192:8240], in_=hpT)
            nc.sync.dma_start(out=dbg[h, 0:2, 8240:10288], in_=s2)

        # ---- main loop over j-chunks ----
        av = pav.tile([128, N1], FP, tag="av")
        # 4 per-i-group row-sum accumulators, packed two per PSUM bank at
        # the legal matmul output partition bases (0 and 32).
        sumpA = psums.tile([33, 512], FP, tag="sumpA")
        sumpB = psums.tile([33, 512], FP, tag="sumpB")

        def sum_slot(g):
            t = sumpA if g < 2 else sumpB
            base = 32 * (g % 2)
            return t[base:base + 1, :]

        for jc in range(16):
            eT = etp.tile([128, N1], BF, tag="eT")
            if TS_FUSED:
                nc.vector.tensor_scalar(eT, rb, v_col[:, jc:jc + 1],
                                        v2_col[:, jc:jc + 1],
                                        op0=ALU.mult, op1=ALU.max)
            else:
                rv = etp.tile([128, N1], FP, tag="rv")
                nc.vector.tensor_scalar(rv, rb, v_col[:, jc:jc + 1], None,
                                        op0=ALU.mult)
                nc.vector.tensor_scalar(eT, rv, v2_col[:, jc:jc + 1], None,
                                        op0=ALU.max)
            if STAGE == 3 and jc == 15:
                for g in range(4):
                    ecp = outp.tile([128, 512], FP, tag="outF")
                    nc.vector.tensor_copy(ecp, eT[:, g5(g)])
                    nc.sync.dma_start(
                        out=dbg[h, :, 2048 + 512 * g: 2048 + 512 * (g + 1)],
                        in_=ecp)
            for g in range(4):
                nc.tensor.matmul(av[:, g5(g)], V[:, c128(jc)], eT[:, g5(g)],
                                 start=(jc == 0), stop=(jc == 15),
                                 skip_group_check=True)
            for g in range(4):
                nc.tensor.matmul(sum_slot(g), ones_col_bf, eT[:, g5(g)],
                                 start=(jc == 0), stop=(jc == 15),
                                 skip_group_check=True)

        # ---- export unnormalized av and the sums; host divides ----
        sum_sb = headp.tile([1, N1], FP, tag="sum_sb")
        for g in range(4):
            nc.vector.tensor_copy(sum_sb[:, g5(g)], sum_slot(g))
        nc.sync.dma_start(out=sums[h, :], in_=sum_sb)
        for g in range(4):
            outF = outp.tile([128, 512], FP, tag="outF")
            nc.vector.tensor_copy(outF, av[:, g5(g)])
            nc.sync.dma_start(out=outT[h, :, g5(g)], in_=outF)


_NC_CACHE = None


def _get_nc():
    global _NC_CACHE
    if _NC_CACHE is None:
        nc = _build()
        nc.finalize()
        _NC_CACHE = nc
    return _NC_CACHE


def make_in_maps(x, prior_feature, x_mask, W_lin, w_head, a_src, a_dst):
    x = np.ascontiguousarray(np.asarray(x, np.float32))
    prior_feature = np.ascontiguousarray(np.asarray(prior_feature, np.float32))
    x_mask_u8 = np.ascontiguousarray(np.asarray(x_mask).astype(np.uint8))
    W_lin = np.ascontiguousarray(np.asarray(W_lin, np.float32))
    w_head = np.ascontiguousarray(np.asarray(w_head, np.float32))
    a_src = np.ascontiguousarray(np.asarray(a_src, np.float32))
    a_dst = np.ascontiguousarray(np.asarray(a_dst, np.float32))
    in_maps = []
    for c in range(NCORES):
        b, h0 = c // 2, (c % 2) * HPC
        in_maps.append(dict(
            x_b=x[b],
            prior_b=prior_feature[b],
            mask_b=x_mask_u8[b],
            W_lin=W_lin,
            w_pair=np.ascontiguousarray(w_head[h0:h0 + HPC]),
            a_src_p=np.ascontiguousarray(a_src[h0:h0 + HPC]),
            a_dst_p=np.ascontiguousarray(a_dst[h0:h0 + HPC]),
        ))
    return in_maps


def combine_results(results, x_mask, bias):
    x_mask = np.asarray(x_mask).astype(bool)
    out = np.zeros((4, N1, O), np.float32)
    for c in range(NCORES):
        b = c // 2
        o = results[c]["outT"]     # [HPC, O, N1] unnormalized
        s = results[c]["sums"]     # [HPC, N1] softmax denominators
        vb = results[c]["vbar_out"]  # [HPC, O] masked-row fill value
        m = x_mask[b]
        acc = np.zeros((O, N1), np.float32)
        for k in range(HPC):
            oh = o[k] / s[k][None, :]
            oh[:, m] = vb[k][:, None]
            acc += oh
        out[b] += acc.T * 0.25
    out += np.asarray(bias, np.float32)[None, None, :]
    return out


def kernel(x, prior_feature, x_mask, W_lin, w_head, a_src, a_dst, bias,
           **run_kwargs):
    from concourse.bass_utils import run_bass_kernel_spmd
    nc = _get_nc()
    in_maps = make_in_maps(x, prior_feature, x_mask, W_lin, w_head,
                           a_src, a_dst)
    br = run_bass_kernel_spmd(nc, in_maps, core_ids=list(range(NCORES)),
                              **run_kwargs)
    out = combine_results(br.results, x_mask, bias)
    if run_kwargs:
        kernel.last_bass_results = br
    return out


# revision 16
# speedup vs baseline: 1.6466x; 1.0439x over previous
"""GAT layer kernel for Trainium2, SPMD over 8 NeuronCores.

Reference computation (per batch b):
  h  = x @ W_lin.T                          [N, O]
  hp = concat(h, prior[None, :])            [N1, O]
  per head: hp_h = hp @ w_head[h]           [N1, O]
  t = tanh(hp_h); s_src = t @ a_src[h]; s_dst = t @ a_dst[h]
  z[i,j] = s_src[i] + s_dst[j]; y = leaky_relu(z, 0.2)
  y[mask_i | mask_j] = -1e18; p = softmax_j(y)
  out_h = p @ hp_h;  out = mean_h(out_h) + bias

Sharding: core c handles batch b=c//2 and heads h in {2*(c%2), 2*(c%2)+1}.

Key algebra: exp(leaky_relu(z)) = max(exp(z), exp(0.2 z)) and z factors as
s_src[i] + s_dst[j].  Softmax is invariant to any positive per-i scale, so
dividing by exp(0.2 s_src[i]) gives the unnormalized weights
  e[j,i] = max(r[i] * v[j], v2[j])
with r = exp(0.8 s_src'), v = exp(s_dst'), v2 = exp(0.2 s_dst'), where the
primed scores carry the mask sentinel (-400, clamped to the exp-table
range).  Each 128-row j-chunk of e is ONE fused DVE tensor_scalar
(mult, max) over bf16 operands producing bf16 weights; the PE accumulates
  avT[o, i] = sum_j V[j, o] e[j, i]   and   sums[i] = sum_j e[j, i]
flash-style in PSUM.  The host divides, fixes fully-masked rows i (whole
row masked -> reference softmax is uniform -> out row = mean_n hp_h = vbar,
exported per head), averages heads, transposes, adds bias.

Engine budget: PE streams e twice (av + sums) in bf16; score matmuls run
in float32r (TF32-like, 1 cycle/row); V / rb matmuls in bf16; all
PSUM->SBUF copies run on the otherwise-idle ACT engine; DVE does only the
fused e ops and small vector work.  Both heads' prologues are issued
before either head's j-loop so the serial score->columns chain of head 1
hides under head 0's main loop.
"""

import sys

for _p in ("/opt/trn_rl_repo",):
    if _p not in sys.path:
        sys.path.insert(0, _p)

import os as _os

import numpy as np

import concourse.bass as bass
import concourse.tile as tile
from concourse import bacc, mybir
from concourse.masks import make_identity

FP = mybir.dt.float32
FPR = mybir.dt.float32r
BF = mybir.dt.bfloat16
U8 = mybir.dt.uint8
N, N1, I, O = 2047, 2048, 256, 128
HPC = 2  # heads per core
NCORES = 8
# Mask sentinel: s' = s - 400 for masked nodes.  exp-table inputs are kept
# inside ~[-87, 88]: v2 = exp(0.2 s') >= exp(-82) needs no clamp; v and r
# inputs are clamped at CLO.  Clamped weights are ~e^-86 ~ 4e-38, vs >=
# ~e^-16 for any live entry -- negligible.
NEG = -400.0
CLO = -86.0
Tanh = mybir.ActivationFunctionType.Tanh
Exp = mybir.ActivationFunctionType.Exp
Ident = mybir.ActivationFunctionType.Identity
ALU = mybir.AluOpType

USE_FPR = not bool(_os.environ.get("GATV3_NO_FPR"))
MMDT = FPR if USE_FPR else FP
STAGE = int(_os.environ.get("GATV3_STAGE", "99"))


def g5(g):
    return slice(g * 512, (g + 1) * 512)


def c128(c):
    return slice(c * 128, (c + 1) * 128)


def _build() -> bass.Bass:
    nc = bacc.Bacc(None, target_bir_lowering=False, debug=False)
    x_b = nc.dram_tensor("x_b", [N, I], FP, kind="ExternalInput")
    prior_b = nc.dram_tensor("prior_b", [O], FP, kind="ExternalInput")
    mask_b = nc.dram_tensor("mask_b", [N1], U8, kind="ExternalInput")
    W_lin = nc.dram_tensor("W_lin", [O, I], FP, kind="ExternalInput")
    w_pair = nc.dram_tensor("w_pair", [HPC, O, O], FP, kind="ExternalInput")
    a_src_p = nc.dram_tensor("a_src_p", [HPC, O], FP, kind="ExternalInput")
    a_dst_p = nc.dram_tensor("a_dst_p", [HPC, O], FP, kind="ExternalInput")
    outT = nc.dram_tensor("outT", [HPC, O, N1], FP, kind="ExternalOutput")
    sums = nc.dram_tensor("sums", [HPC, N1], FP, kind="ExternalOutput")
    vbar_out = nc.dram_tensor("vbar_out", [HPC, O], FP, kind="ExternalOutput")

    with tile.TileContext(nc) as tc:
        with (
            tc.tile_pool(name="constp", bufs=1) as constp,
            tc.tile_pool(name="bigp", bufs=1) as bigp,
            tc.tile_pool(name="headp", bufs=2) as headp,
            tc.tile_pool(name="etp", bufs=5) as etp,
            tc.tile_pool(name="outp", bufs=2) as outp,
            tc.tile_pool(name="pp", bufs=2, space="PSUM") as pp,
            tc.tile_pool(name="pav", bufs=1, space="PSUM") as pav,
            tc.tile_pool(name="psums", bufs=1, space="PSUM") as psums,
        ):
            pools = dict(constp=constp, bigp=bigp, headp=headp, etp=etp,
                         outp=outp, pp=pp, pav=pav, psums=psums, tc=tc)
            _body(nc, tc, pools,
                  x_b, prior_b, mask_b, W_lin, w_pair, a_src_p, a_dst_p,
                  outT, sums, vbar_out)
    return nc


def _body(nc, tc, pools,
          x_b, prior_b, mask_b, W_lin, w_pair, a_src_p, a_dst_p,
          outT, sums, vbar_out):
    constp, bigp, headp = pools["constp"], pools["bigp"], pools["headp"]
    etp, outp = pools["etp"], pools["outp"]
    pp, pav, psums = pools["pp"], pools["pav"], pools["psums"]
    tcx = pools["tc"]

    # ---- constants ----
    ident = constp.tile([128, 128], FP, tag="ident")
    make_identity(nc, ident)
    ones_row_bf = constp.tile([1, 128], BF, tag="ones_row_bf")
    nc.vector.memset(ones_row_bf, 1.0)
    ones_col_bf = constp.tile([128, 1], BF, tag="ones_col_bf")
    nc.vector.memset(ones_col_bf, 1.0)

    # mask as row (for s_src) and as 16 column chunks (for s_dst)
    m_row_u8 = constp.tile([1, N1], U8, tag="m_row_u8")
    nc.sync.dma_start(out=m_row_u8, in_=mask_b[None, :])
    negm_row = constp.tile([1, N1], FP, tag="negm_row")
    nc.vector.tensor_scalar(negm_row, m_row_u8, NEG, None, op0=ALU.mult)
    m_col_u8 = constp.tile([128, 16], U8, tag="m_col_u8")
    nc.sync.dma_start(out=m_col_u8,
                      in_=mask_b[:].rearrange("(c p) -> p c", p=128))
    negm_col = constp.tile([128, 16], FP, tag="negm_col")
    nc.vector.tensor_scalar(negm_col, m_col_u8, NEG, None, op0=ALU.mult)

    # hp^T in float32r (score matmuls) and bf16 (V matmuls)
    hpT_r = bigp.tile([128, N1], MMDT, tag="hpT_r")
    hpT_bf = bigp.tile([128, N1], BF, tag="hpT_bf")
    with tcx.tile_pool(name="prep", bufs=1) as prep:
        # ---- W_lin transposed: wlT[:, k, :] = W_lin[:, k*128:...].T ----
        wl = prep.tile([128, I], FP, tag="wl", bufs=1)
        nc.sync.dma_start(out=wl, in_=W_lin[:, :])
        wlT = prep.tile([128, 2, 128], MMDT, tag="wlT", bufs=1)
        for k in range(2):
            ps = pp.tile([128, 512], FP, tag="tr")
            nc.tensor.transpose(ps[:, :128], wl[:, c128(k)], ident)
            nc.vector.tensor_copy(wlT[:, k, :], ps[:, :128])
        prior_sb = prep.tile([128, 1], FP, tag="prior_sb", bufs=1)
        nc.sync.dma_start(out=prior_sb, in_=prior_b[:, None])

        # ---- x transposed per group, hp matmul issued as soon as its 4
        # tiles are ready (row 2047 zeroed; col 2047 <- prior below) ----
        xT = prep.tile([128, 2, N1], MMDT, tag="xT", bufs=1)
        for g in range(4):
            for t in range(4 * g, 4 * g + 4):
                rows = 128 if t < 15 else 127
                xn = prep.tile([128, I], FP, tag="xn", bufs=3)
                if rows < 128:
                    nc.vector.memset(xn, 0.0)
                nc.sync.dma_start(out=xn[:rows, :],
                                  in_=x_b[t * 128: t * 128 + rows, :])
                for k in range(2):
                    ps = pp.tile([128, 512], FP, tag="tr")
                    nc.tensor.transpose(ps[:, :128], xn[:, c128(k)], ident)
                    nc.vector.tensor_copy(xT[:, k, t * 128: (t + 1) * 128],
                                          ps[:, :128])
            ph = pp.tile([128, 512], FP, tag="tr")
            for k in range(2):
                nc.tensor.matmul(ph, wlT[:, k, :], xT[:, k, g5(g)],
                                 start=(k == 0), stop=(k == 1))
            nc.scalar.activation(hpT_r[:, g5(g)], ph, Ident)
            nc.scalar.activation(hpT_bf[:, g5(g)], ph, Ident)
        nc.scalar.activation(hpT_r[:, 2047:2048], prior_sb, Ident)
        nc.scalar.activation(hpT_bf[:, 2047:2048], prior_sb, Ident)

    # column sums of hp (for vbar = (hpbar @ w_head) / N1)
    hpbar_col = constp.tile([128, 1], FP, tag="hpbar_col")
    nc.vector.reduce_sum(hpbar_col, hpT_r[:, :].bitcast(FP),
                         axis=mybir.AxisListType.X)

    if STAGE == 1:
        nc.sync.dma_start(out=outT[0, :, :], in_=hpT_r[:, :].bitcast(FP))
        return

    # ================= phase A: per-head prologues =================
    H = {}
    for h in range(HPC):
        wh = headp.tile([128, 128], FP, tag="wh")
        nc.sync.dma_start(out=wh, in_=w_pair[h])
        wh_r = headp.tile([128, 128], MMDT, tag="wh_r")
        nc.scalar.activation(wh_r, wh, Ident)
        wh_bf = headp.tile([128, 128], BF, tag="wh_bf")
        nc.scalar.activation(wh_bf, wh, Ident)
        acols = headp.tile([128, 2], FP, tag="acols")
        nc.sync.dma_start(out=acols[:, 0:1], in_=a_src_p[h][:, None])
        nc.sync.dma_start(out=acols[:, 1:2], in_=a_dst_p[h][:, None])
        acols_r = headp.tile([128, 2], MMDT, tag="acols_r")
        nc.scalar.activation(acols_r, acols, Ident)

        # tanh(hp_h^T)
        tT = headp.tile([128, N1], MMDT, tag="tT")
        for g in range(4):
            php = pp.tile([128, 512], FP, tag="tr")
            nc.tensor.matmul(php, wh_r, hpT_r[:, g5(g)], start=True, stop=True)
            nc.scalar.activation(tT[:, g5(g)], php, Tanh)

        # s_src' row: a_src^T @ tT + NEG*mask (M=2 form: fp32r requires it)
        srow = headp.tile([1, N1], FP, tag="srow")
        for g in range(4):
            ps1 = pp.tile([128, 512], FP, tag="tr")
            nc.tensor.matmul(ps1[:2, :], acols_r, tT[:, g5(g)],
                             start=True, stop=True)
            nc.vector.tensor_tensor(srow[:, g5(g)], ps1[:1, :],
                                    negm_row[:, g5(g)], op=ALU.add)

        # s_dst' columns: 16 single-column matmuls tT_chunk^T @ a_dst
        psd = pp.tile([128, 512], FP, tag="tr")
        for c in range(16):
            nc.tensor.matmul(psd[:, c:c + 1],
                             tT[:, c128(c)].bitcast(FP) if USE_FPR
                             else tT[:, c128(c)],
                             acols[:, 1:2], start=True, stop=True,
                             skip_group_check=True)
        sdc = headp.tile([128, 16], FP, tag="sdc")
        nc.vector.tensor_tensor(sdc, psd[:, 0:16], negm_col, op=ALU.add)
        sdc_c = headp.tile([128, 16], FP, tag="sdc_c")
        nc.vector.tensor_scalar_max(sdc_c, sdc, CLO)
        v_col = headp.tile([128, 16], FP, tag="v_col")
        nc.scalar.activation(v_col, sdc_c, Exp)
        # 0.2 * sdc >= -82: already inside the exp table range
        v2_col = headp.tile([128, 16], FP, tag="v2_col")
        nc.scalar.activation(v2_col, sdc, Exp, scale=0.2)

        # r = exp(0.8 s_src') broadcast to all partitions via PE (bf16)
        rr_tmp = headp.tile([1, N1], FP, tag="rr_tmp")
        nc.vector.tensor_scalar(rr_tmp, srow, 0.8, CLO,
                                op0=ALU.mult, op1=ALU.max)
        r_row = headp.tile([1, N1], BF, tag="r_row")
        nc.scalar.activation(r_row, rr_tmp, Exp)
        rb = headp.tile([128, N1], BF, tag="rb")
        for g in range(4):
            prb = pp.tile([128, 512], FP, tag="tr")
            nc.tensor.matmul(prb, ones_row_bf, r_row[:, g5(g)],
                             start=True, stop=True)
            nc.scalar.activation(rb[:, g5(g)], prb, Ident)

        # V = hp_h natural [n, p] (bf16) and vbar
        V = headp.tile([128, N1], BF, tag="V")
        for t in range(16):
            pv = pp.tile([128, 512], FP, tag="tr")
            nc.tensor.matmul(pv[:, :128], hpT_bf[:, c128(t)], wh_bf,
                             start=True, stop=True)
            nc.scalar.activation(V[:, c128(t)], pv[:, :128], Ident)
        pvb = pp.tile([128, 512], FP, tag="tr")
        nc.tensor.matmul(pvb[:1, :128], hpbar_col, wh, start=True, stop=True)
        vbar_sb = headp.tile([1, 128], FP, tag="vbar_sb")
        nc.vector.tensor_scalar_mul(vbar_sb, pvb[:1, :128], 1.0 / N1)
        nc.sync.dma_start(out=vbar_out[h, :], in_=vbar_sb)

        H[h] = dict(V=V, rb=rb, v_col=v_col, v2_col=v2_col)

    if STAGE == 2:
        nc.sync.dma_start(out=outT[0, :, :], in_=hpT_r[:, :].bitcast(FP))
        return

    # ================= phase B: per-head j-loops =================
    for h in range(HPC):
        V, rb = H[h]["V"], H[h]["rb"]
        v_col, v2_col = H[h]["v_col"], H[h]["v2_col"]

        av = pav.tile([128, N1], FP, tag="av")
        # 4 per-i-group row-sum accumulators, packed two per PSUM bank at
        # the legal matmul output partition bases (0 and 32).
        sumpA = psums.tile([33, 512], FP, tag="sumpA")
        sumpB = psums.tile([33, 512], FP, tag="sumpB")

        def sum_slot(g):
            t = sumpA if g < 2 else sumpB
            base = 32 * (g % 2)
            return t[base:base + 1, :]

        for jc in range(16):
            eT = etp.tile([128, N1], BF, tag="eT")
            nc.vector.tensor_scalar(eT, rb, v_col[:, jc:jc + 1],
                                    v2_col[:, jc:jc + 1],
                                    op0=ALU.mult, op1=ALU.max)
            for g in range(4):
                nc.tensor.matmul(av[:, g5(g)], V[:, c128(jc)], eT[:, g5(g)],
                                 start=(jc == 0), stop=(jc == 15),
                                 skip_group_check=True)
            for g in range(4):
                nc.tensor.matmul(sum_slot(g), ones_col_bf, eT[:, g5(g)],
                                 start=(jc == 0), stop=(jc == 15),
                                 skip_group_check=True)

        # ---- export unnormalized av and the sums; host divides ----
        sum_sb = headp.tile([1, N1], FP, tag="sum_sb")
        for g in range(4):
            nc.scalar.activation(sum_sb[:, g5(g)], sum_slot(g), Ident)
        nc.sync.dma_start(out=sums[h, :], in_=sum_sb)
        for g in range(4):
            outF = outp.tile([128, 512], FP, tag="outF")
            nc.scalar.activation(outF, av[:, g5(g)], Ident)
            nc.sync.dma_start(out=outT[h, :, g5(g)], in_=outF)


_NC_CACHE = None


def _get_nc():
    global _NC_CACHE
    if _NC_CACHE is None:
        nc = _build()
        nc.finalize()
        _NC_CACHE = nc
    return _NC_CACHE


def make_in_maps(x, prior_feature, x_mask, W_lin, w_head, a_src, a_dst):
    x = np.ascontiguousarray(np.asarray(x, np.float32))
    prior_feature = np.ascontiguousarray(np.asarray(prior_feature, np.float32))
    x_mask_u8 = np.ascontiguousarray(np.asarray(x_mask).astype(np.uint8))
    W_lin = np.ascontiguousarray(np.asarray(W_lin, np.float32))
    w_head = np.ascontiguousarray(np.asarray(w_head, np.float32))
    a_src = np.ascontiguousarray(np.asarray(a_src, np.float32))
    a_dst = np.ascontiguousarray(np.asarray(a_dst, np.float32))
    in_maps = []
    for c in range(NCORES):
        b, h0 = c // 2, (c % 2) * HPC
        in_maps.append(dict(
            x_b=x[b],
            prior_b=prior_feature[b],
            mask_b=x_mask_u8[b],
            W_lin=W_lin,
            w_pair=np.ascontiguousarray(w_head[h0:h0 + HPC]),
            a_src_p=np.ascontiguousarray(a_src[h0:h0 + HPC]),
            a_dst_p=np.ascontiguousarray(a_dst[h0:h0 + HPC]),
        ))
    return in_maps


def combine_results(results, x_mask, bias):
    x_mask = np.asarray(x_mask).astype(bool)
    out = np.zeros((4, N1, O), np.float32)
    for c in range(NCORES):
        b = c // 2
        o = results[c]["outT"]       # [HPC, O, N1] unnormalized
        s = results[c]["sums"]       # [HPC, N1] softmax denominators
        vb = results[c]["vbar_out"]  # [HPC, O] masked-row fill value
        m = x_mask[b]
        acc = np.zeros((O, N1), np.float32)
        for k in range(HPC):
            oh = o[k] / s[k][None, :]
            oh[:, m] = vb[k][:, None]
            acc += oh
        out[b] += acc.T * 0.25
    out += np.asarray(bias, np.float32)[None, None, :]
    return out


def kernel(x, prior_feature, x_mask, W_lin, w_head, a_src, a_dst, bias,
           **run_kwargs):
    from concourse.bass_utils import run_bass_kernel_spmd
    nc = _get_nc()
    in_maps = make_in_maps(x, prior_feature, x_mask, W_lin, w_head,
                           a_src, a_dst)
    br = run_bass_kernel_spmd(nc, in_maps, core_ids=list(range(NCORES)),
                              **run_kwargs)
    out = combine_results(br.results, x_mask, bias)
    if run_kwargs:
        kernel.last_bass_results = br
    return out


# revision 18
# speedup vs baseline: 1.7165x; 1.0424x over previous
"""GAT layer kernel for Trainium2, SPMD over 8 NeuronCores.

Reference computation (per batch b):
  h  = x @ W_lin.T                          [N, O]
  hp = concat(h, prior[None, :])            [N1, O]
  per head: hp_h = hp @ w_head[h]           [N1, O]
  t = tanh(hp_h); s_src = t @ a_src[h]; s_dst = t @ a_dst[h]
  z[i,j] = s_src[i] + s_dst[j]; y = leaky_relu(z, 0.2)
  y[mask_i | mask_j] = -1e18; p = softmax_j(y)
  out_h = p @ hp_h;  out = mean_h(out_h) + bias

Sharding: core c handles batch b=c//2 and heads h in {2*(c%2), 2*(c%2)+1}.

Key algebra: exp(leaky_relu(z)) = max(exp(z), exp(0.2 z)) and z factors as
s_src[i] + s_dst[j].  Softmax is invariant to any positive per-i scale, so
dividing by exp(0.2 s_src[i]) gives the unnormalized weights
  e[j,i] = max(r[i] * v[j], v2[j])
with r = exp(0.8 s_src'), v = exp(s_dst'), v2 = exp(0.2 s_dst'), where the
primed scores carry the mask sentinel (-400, clamped to the exp-table
range).  Each 128-row j-chunk of e is ONE fused DVE tensor_scalar
(mult, max) over bf16 operands producing bf16 weights; the PE accumulates
  avT[o, i] = sum_j V[j, o] e[j, i]   and   sums[i] = sum_j e[j, i]
flash-style in PSUM.  The host divides, fixes fully-masked rows i (whole
row masked -> reference softmax is uniform -> out row = mean_n hp_h = vbar,
exported per head), averages heads, transposes, adds bias.

Engine budget: PE streams e twice (av + sums) in bf16; score matmuls run
in float32r (TF32-like, 1 cycle/row); V / rb matmuls in bf16; all
PSUM->SBUF copies run on the otherwise-idle ACT engine; DVE does only the
fused e ops and small vector work.  Both heads' prologues are issued
before either head's j-loop so the serial score->columns chain of head 1
hides under head 0's main loop.
"""

import sys

for _p in ("/opt/trn_rl_repo",):
    if _p not in sys.path:
        sys.path.insert(0, _p)

import os as _os

import numpy as np

import concourse.bass as bass
import concourse.tile as tile
from concourse import bacc, mybir
from concourse.masks import make_identity

FP = mybir.dt.float32
FPR = mybir.dt.float32r
BF = mybir.dt.bfloat16
U8 = mybir.dt.uint8
N, N1, I, O = 2047, 2048, 256, 128
HPC = 2  # heads per core
NCORES = 8
# Mask sentinel: s' = s - 400 for masked nodes.  exp-table inputs are kept
# inside ~[-87, 88]: v2 = exp(0.2 s') >= exp(-82) needs no clamp; v and r
# inputs are clamped at CLO.  Clamped weights are ~e^-86 ~ 4e-38, vs >=
# ~e^-16 for any live entry -- negligible.
NEG = -400.0
CLO = -86.0
Tanh = mybir.ActivationFunctionType.Tanh
Exp = mybir.ActivationFunctionType.Exp
Ident = mybir.ActivationFunctionType.Identity
ALU = mybir.AluOpType

USE_FPR = not bool(_os.environ.get("GATV3_NO_FPR"))
MMDT = FPR if USE_FPR else FP
STAGE = int(_os.environ.get("GATV3_STAGE", "99"))


def g5(g):
    return slice(g * 512, (g + 1) * 512)


def c128(c):
    return slice(c * 128, (c + 1) * 128)


def _build() -> bass.Bass:
    nc = bacc.Bacc(None, target_bir_lowering=False, debug=False)
    x_b = nc.dram_tensor("x_b", [N, I], FP, kind="ExternalInput")
    prior_b = nc.dram_tensor("prior_b", [O], FP, kind="ExternalInput")
    mask_b = nc.dram_tensor("mask_b", [N1], U8, kind="ExternalInput")
    W_lin = nc.dram_tensor("W_lin", [O, I], FP, kind="ExternalInput")
    w_pair = nc.dram_tensor("w_pair", [HPC, O, O], FP, kind="ExternalInput")
    a_src_p = nc.dram_tensor("a_src_p", [HPC, O], FP, kind="ExternalInput")
    a_dst_p = nc.dram_tensor("a_dst_p", [HPC, O], FP, kind="ExternalInput")
    outT = nc.dram_tensor("outT", [HPC, O, N1], FP, kind="ExternalOutput")
    sums = nc.dram_tensor("sums", [HPC, N1], FP, kind="ExternalOutput")
    vbar_out = nc.dram_tensor("vbar_out", [HPC, O], FP, kind="ExternalOutput")
    sdst_dram = nc.dram_tensor("sdst_scratch", [HPC, N1], FP)

    with tile.TileContext(nc) as tc:
        with (
            tc.tile_pool(name="constp", bufs=1) as constp,
            tc.tile_pool(name="bigp", bufs=1) as bigp,
            tc.tile_pool(name="headp", bufs=2) as headp,
            tc.tile_pool(name="etp", bufs=5) as etp,
            tc.tile_pool(name="outp", bufs=2) as outp,
            tc.tile_pool(name="pp", bufs=2, space="PSUM") as pp,
            tc.tile_pool(name="pav", bufs=1, space="PSUM") as pav,
            tc.tile_pool(name="psums", bufs=1, space="PSUM") as psums,
        ):
            pools = dict(constp=constp, bigp=bigp, headp=headp, etp=etp,
                         outp=outp, pp=pp, pav=pav, psums=psums, tc=tc)
            _body(nc, tc, pools,
                  x_b, prior_b, mask_b, W_lin, w_pair, a_src_p, a_dst_p,
                  outT, sums, vbar_out, sdst_dram)
    return nc


def _body(nc, tc, pools,
          x_b, prior_b, mask_b, W_lin, w_pair, a_src_p, a_dst_p,
          outT, sums, vbar_out, sdst_dram):
    constp, bigp, headp = pools["constp"], pools["bigp"], pools["headp"]
    etp, outp = pools["etp"], pools["outp"]
    pp, pav, psums = pools["pp"], pools["pav"], pools["psums"]
    tcx = pools["tc"]

    # ---- constants ----
    ident = constp.tile([128, 128], FP, tag="ident")
    make_identity(nc, ident)
    ones_row_bf = constp.tile([1, 128], BF, tag="ones_row_bf")
    nc.vector.memset(ones_row_bf, 1.0)
    ones_col_bf = constp.tile([128, 1], BF, tag="ones_col_bf")
    nc.vector.memset(ones_col_bf, 1.0)

    # mask as row (for s_src) and as 16 column chunks (for s_dst)
    m2_u8 = constp.tile([2, N1], U8, tag="m2_u8")
    nc.sync.dma_start(out=m2_u8[0:1, :], in_=mask_b[None, :])
    nc.sync.dma_start(out=m2_u8[1:2, :], in_=mask_b[None, :])
    negm2 = constp.tile([2, N1], FP, tag="negm2")
    nc.vector.tensor_scalar(negm2, m2_u8, NEG, None, op0=ALU.mult)

    # ---- both heads' weights: DMA + casts issued before everything ----
    HW_ = {}
    for h in range(HPC):
        wh = headp.tile([128, 128], FP, tag="wh")
        nc.sync.dma_start(out=wh, in_=w_pair[h])
        wh_r = headp.tile([128, 128], MMDT, tag="wh_r")
        nc.scalar.activation(wh_r, wh, Ident)
        wh_bf = headp.tile([128, 128], BF, tag="wh_bf")
        nc.scalar.activation(wh_bf, wh, Ident)
        acols = headp.tile([128, 2], FP, tag="acols")
        nc.sync.dma_start(out=acols[:, 0:1], in_=a_src_p[h][:, None])
        nc.sync.dma_start(out=acols[:, 1:2], in_=a_dst_p[h][:, None])
        acols_r = headp.tile([128, 2], MMDT, tag="acols_r")
        nc.scalar.activation(acols_r, acols, Ident)
        HW_[h] = dict(wh=wh, wh_r=wh_r, wh_bf=wh_bf, acols=acols,
                      acols_r=acols_r)

    # hp^T in float32r (score matmuls) and bf16 (V matmuls)
    hpT_r = bigp.tile([128, N1], MMDT, tag="hpT_r")
    hpT_bf = bigp.tile([128, N1], BF, tag="hpT_bf")
    with tcx.tile_pool(name="prep", bufs=1) as prep:
        # ---- W_lin transposed: wlT[:, k, :] = W_lin[:, k*128:...].T ----
        wl = prep.tile([128, I], FP, tag="wl", bufs=1)
        nc.sync.dma_start(out=wl, in_=W_lin[:, :])
        wlT = prep.tile([128, 2, 128], MMDT, tag="wlT", bufs=1)
        for k in range(2):
            ps = pp.tile([128, 512], FP, tag="tr")
            nc.tensor.transpose(ps[:, :128], wl[:, c128(k)], ident)
            nc.vector.tensor_copy(wlT[:, k, :], ps[:, :128])
        prior_sb = prep.tile([128, 1], FP, tag="prior_sb", bufs=1)
        nc.sync.dma_start(out=prior_sb, in_=prior_b[:, None])

        # ---- x transposed per group, hp matmul issued as soon as its 4
        # tiles are ready (row 2047 zeroed; col 2047 <- prior below) ----
        xT = prep.tile([128, 2, N1], MMDT, tag="xT", bufs=1)
        for g in range(4):
            for t in range(4 * g, 4 * g + 4):
                rows = 128 if t < 15 else 127
                xn = prep.tile([128, I], FP, tag="xn", bufs=3)
                if rows < 128:
                    nc.vector.memset(xn, 0.0)
                nc.sync.dma_start(out=xn[:rows, :],
                                  in_=x_b[t * 128: t * 128 + rows, :])
                for k in range(2):
                    ps = pp.tile([128, 512], FP, tag="tr")
                    nc.tensor.transpose(ps[:, :128], xn[:, c128(k)], ident)
                    nc.vector.tensor_copy(xT[:, k, t * 128: (t + 1) * 128],
                                          ps[:, :128])
            ph = pp.tile([128, 512], FP, tag="tr")
            for k in range(2):
                nc.tensor.matmul(ph, wlT[:, k, :], xT[:, k, g5(g)],
                                 start=(k == 0), stop=(k == 1))
            nc.scalar.activation(hpT_r[:, g5(g)], ph, Ident)
            nc.scalar.activation(hpT_bf[:, g5(g)], ph, Ident)
        nc.scalar.activation(hpT_r[:, 2047:2048], prior_sb, Ident)
        nc.scalar.activation(hpT_bf[:, 2047:2048], prior_sb, Ident)

    # column sums of hp (for vbar = (hpbar @ w_head) / N1)
    hpbar_col = constp.tile([128, 1], FP, tag="hpbar_col")
    nc.vector.reduce_sum(hpbar_col, hpT_r[:, :].bitcast(FP),
                         axis=mybir.AxisListType.X)

    if STAGE == 1:
        nc.sync.dma_start(out=outT[0, :, :], in_=hpT_r[:, :].bitcast(FP))
        return

    # ================= phase A1: scores / V / bounce =================
    H = {}
    for h in range(HPC):
        wh, wh_r, wh_bf = HW_[h]["wh"], HW_[h]["wh_r"], HW_[h]["wh_bf"]
        acols_r = HW_[h]["acols_r"]

        # tanh(hp_h^T)
        tT = headp.tile([128, N1], MMDT, tag="tT")
        for g in range(4):
            php = pp.tile([128, 512], FP, tag="tr")
            nc.tensor.matmul(php, wh_r, hpT_r[:, g5(g)], start=True, stop=True)
            nc.scalar.activation(tT[:, g5(g)], php, Tanh)

        # s' rows: [a_src | a_dst]^T @ tT + NEG*mask
        s2row = headp.tile([2, N1], FP, tag="s2row")
        for g in range(4):
            ps1 = pp.tile([128, 512], FP, tag="tr")
            nc.tensor.matmul(ps1[:2, :], acols_r, tT[:, g5(g)],
                             start=True, stop=True)
            nc.vector.tensor_tensor(s2row[:, g5(g)], ps1[:2, :],
                                    negm2[:, g5(g)], op=ALU.add)
        # s_dst' to column layout via DRAM bounce (latency hidden: consumed
        # only at this head's j-loop)
        nc.sync.dma_start(out=sdst_dram[h, :], in_=s2row[1:2, :])
        sdc = headp.tile([128, 16], FP, tag="sdc")
        nc.sync.dma_start(out=sdc,
                          in_=sdst_dram[h, :].rearrange("(c p) -> p c", p=128))

        # V = hp_h natural [n, p] (bf16) and vbar
        V = headp.tile([128, N1], BF, tag="V")
        for t in range(16):
            pv = pp.tile([128, 512], FP, tag="tr")
            nc.tensor.matmul(pv[:, :128], hpT_bf[:, c128(t)], wh_bf,
                             start=True, stop=True)
            nc.scalar.activation(V[:, c128(t)], pv[:, :128], Ident)
        pvb = pp.tile([128, 512], FP, tag="tr")
        nc.tensor.matmul(pvb[:1, :128], hpbar_col, wh, start=True, stop=True)
        vbar_sb = headp.tile([1, 128], FP, tag="vbar_sb")
        nc.vector.tensor_scalar_mul(vbar_sb, pvb[:1, :128], 1.0 / N1)
        nc.sync.dma_start(out=vbar_out[h, :], in_=vbar_sb)

        H[h] = dict(V=V, s2row=s2row, sdc=sdc)

    # ================= phase A2: r broadcast + column exps =================
    for h in range(HPC):
        s2row, sdc = H[h]["s2row"], H[h]["sdc"]
        sdc_c = headp.tile([128, 16], FP, tag="sdc_c")
        nc.vector.tensor_scalar_max(sdc_c, sdc, CLO)
        v_col = headp.tile([128, 16], FP, tag="v_col")
        nc.scalar.activation(v_col, sdc_c, Exp)
        # 0.2 * sdc >= -82: already inside the exp table range
        v2_col = headp.tile([128, 16], FP, tag="v2_col")
        nc.scalar.activation(v2_col, sdc, Exp, scale=0.2)

        rr_tmp = headp.tile([1, N1], FP, tag="rr_tmp")
        nc.vector.tensor_scalar(rr_tmp, s2row[0:1, :], 0.8, CLO,
                                op0=ALU.mult, op1=ALU.max)
        r_row = headp.tile([1, N1], BF, tag="r_row")
        nc.scalar.activation(r_row, rr_tmp, Exp)
        rb = headp.tile([128, N1], BF, tag="rb")
        for g in range(4):
            prb = pp.tile([128, 512], FP, tag="tr")
            nc.tensor.matmul(prb, ones_row_bf, r_row[:, g5(g)],
                             start=True, stop=True)
            nc.scalar.activation(rb[:, g5(g)], prb, Ident)
        H[h].update(rb=rb, v_col=v_col, v2_col=v2_col)

    if STAGE == 2:
        nc.sync.dma_start(out=outT[0, :, :], in_=hpT_r[:, :].bitcast(FP))
        return

    # ================= phase B: per-head j-loops =================
    for h in range(HPC):
        V, rb = H[h]["V"], H[h]["rb"]
        v_col, v2_col = H[h]["v_col"], H[h]["v2_col"]

        av = pav.tile([128, N1], FP, tag="av")
        # 4 per-i-group row-sum accumulators, packed two per PSUM bank at
        # the legal matmul output partition bases (0 and 32).
        sumpA = psums.tile([33, 512], FP, tag="sumpA")
        sumpB = psums.tile([33, 512], FP, tag="sumpB")

        def sum_slot(g):
            t = sumpA if g < 2 else sumpB
            base = 32 * (g % 2)
            return t[base:base + 1, :]

        for jc in range(16):
            eT = etp.tile([128, N1], BF, tag="eT")
            nc.vector.tensor_scalar(eT, rb, v_col[:, jc:jc + 1],
                                    v2_col[:, jc:jc + 1],
                                    op0=ALU.mult, op1=ALU.max)
            for g in range(4):
                nc.tensor.matmul(av[:, g5(g)], V[:, c128(jc)], eT[:, g5(g)],
                                 start=(jc == 0), stop=(jc == 15),
                                 skip_group_check=True)
            for g in range(4):
                nc.tensor.matmul(sum_slot(g), ones_col_bf, eT[:, g5(g)],
                                 start=(jc == 0), stop=(jc == 15),
                                 skip_group_check=True)

        # ---- export unnormalized av and the sums; host divides ----
        sum_sb = headp.tile([1, N1], FP, tag="sum_sb")
        for g in range(4):
            if g < 2:
                nc.scalar.activation(sum_sb[:, g5(g)], sum_slot(g), Ident)
            else:
                nc.vector.tensor_copy(sum_sb[:, g5(g)], sum_slot(g))
        nc.sync.dma_start(out=sums[h, :], in_=sum_sb)
        for g in range(4):
            outF = outp.tile([128, 512], FP, tag="outF")
            if g % 2 == 0:
                nc.scalar.activation(outF, av[:, g5(g)], Ident)
            else:
                nc.vector.tensor_copy(outF, av[:, g5(g)])
            nc.sync.dma_start(out=outT[h, :, g5(g)], in_=outF)


_NC_CACHE = None


def _get_nc():
    global _NC_CACHE
    if _NC_CACHE is None:
        nc = _build()
        nc.finalize()
        _NC_CACHE = nc
    return _NC_CACHE


def make_in_maps(x, prior_feature, x_mask, W_lin, w_head, a_src, a_dst):
    x = np.ascontiguousarray(np.asarray(x, np.float32))
    prior_feature = np.ascontiguousarray(np.asarray(prior_feature, np.float32))
    x_mask_u8 = np.ascontiguousarray(np.asarray(x_mask).astype(np.uint8))
    W_lin = np.ascontiguousarray(np.asarray(W_lin, np.float32))
    w_head = np.ascontiguousarray(np.asarray(w_head, np.float32))
    a_src = np.ascontiguousarray(np.asarray(a_src, np.float32))
    a_dst = np.ascontiguousarray(np.asarray(a_dst, np.float32))
    in_maps = []
    for c in range(NCORES):
        b, h0 = c // 2, (c % 2) * HPC
        in_maps.append(dict(
            x_b=x[b],
            prior_b=prior_feature[b],
            mask_b=x_mask_u8[b],
            W_lin=W_lin,
            w_pair=np.ascontiguousarray(w_head[h0:h0 + HPC]),
            a_src_p=np.ascontiguousarray(a_src[h0:h0 + HPC]),
            a_dst_p=np.ascontiguousarray(a_dst[h0:h0 + HPC]),
        ))
    return in_maps


def combine_results(results, x_mask, bias):
    x_mask = np.asarray(x_mask).astype(bool)
    out = np.zeros((4, N1, O), np.float32)
    for c in range(NCORES):
        b = c // 2
        o = results[c]["outT"]       # [HPC, O, N1] unnormalized
        s = results[c]["sums"]       # [HPC, N1] softmax denominators
        vb = results[c]["vbar_out"]  # [HPC, O] masked-row fill value
        m = x_mask[b]
        acc = np.zeros((O, N1), np.float32)
        for k in range(HPC):
            oh = o[k] / s[k][None, :]
            oh[:, m] = vb[k][:, None]
            acc += oh
        out[b] += acc.T * 0.25
    out += np.asarray(bias, np.float32)[None, None, :]
    return out


def kernel(x, prior_feature, x_mask, W_lin, w_head, a_src, a_dst, bias,
           **run_kwargs):
    from concourse.bass_utils import run_bass_kernel_spmd
    nc = _get_nc()
    in_maps = make_in_maps(x, prior_feature, x_mask, W_lin, w_head,
                           a_src, a_dst)
    br = run_bass_kernel_spmd(nc, in_maps, core_ids=list(range(NCORES)),
                              **run_kwargs)
    out = combine_results(br.results, x_mask, bias)
    if run_kwargs:
        kernel.last_bass_results = br
    return out
